# revision 1
# baseline (speedup 1.0000x reference)
"""2-layer GATv2 + global mean pool + linear head, on 8 Trainium2 NeuronCores.

Strategy (dst-sharded, degree-balanced, batched dma_gather):
  - Nodes are relabeled by a degree-balanced bin-packing into groups of <=128
    destination nodes with near-equal incident-edge counts.  Core c owns
    nodes [c*own, (c+1)*own); edges are assigned by destination owner.
  - Edges live in a static chunk grid (128 edge slots per chunk).  Each
    group's chunks are split into 4 source-subrange segments (int16
    dma_gather indices address <=32768 table rows) with per-(group,subrange)
    chunk budgets shared across cores, so one SPMD program fits all cores.
  - Blocks of 3 groups are processed at a time: batched dma_gather calls
    (<=4 chunks each, 4 SWDGE queues) fetch xl[src] (G) from the AllGather'd
    table and xr[dst] (V) from the core-local table; all elementwise work
    (z=G+V, leaky, att-mult, segment-score reduce, exp, p*G) runs as a few
    block-wide vector/ACT ops; a per-chunk PE matmul with the is_equal
    indicator scatters [p*G | p] into per-group PSUM accumulators (f32),
    giving numerators and softmax denominators at once.
  - Layer tables xl/xr = x@W + b are built on device; xl is AllGather'ed.
    Final pooling partial sums are AllReduce'd; every core computes the
    tiny output head.
"""

import sys

for _p in ("/opt/trn_rl_repo",):
    if _p not in sys.path:
        sys.path.insert(0, _p)

import numpy as np
import ml_dtypes

BF = ml_dtypes.bfloat16

import concourse.bass as bass
import concourse.mybir as mybir
from concourse.tile import TileContext
from concourse.bass_utils import run_bass_kernel_spmd
from concourse.masks import make_identity
from concourse import library_config
from concourse.library_overlay import lower_extended_insts

F32 = mybir.dt.float32
BF16 = mybir.dt.bfloat16
I16 = mybir.dt.int16
P = 128
D = 128
NCORES = 8
NUM_GRAPHS = 64
NEG_SLOPE = 0.2
SUBR = 25000          # int16 dma_gather window (4 x 25000 = 100000 rows)
NSUB = 4
GBLK = 2              # groups per processing block
CALL_CHUNKS = 8       # chunks per dma_gather call (1024 rows)


# ---------------------------------------------------------------- prof hook
def _install_profhook():
    """Provide antenv.axon_hooks (absent in this image) so trace=True works."""
    import types

    if "antenv.axon_hooks" in sys.modules:
        return
    try:
        from trn_agent_boot.trn_boot import _ntff_profile_via_ctypes
    except Exception:
        return
    mod = types.ModuleType("antenv.axon_hooks")
    mod._hook = None
    mod.set_axon_ntff_profile_hook = lambda h: setattr(mod, "_hook", h)
    mod.get_axon_ntff_profile_hook = lambda: mod._hook
    sys.modules["antenv.axon_hooks"] = mod
    try:
        mod._hook = _ntff_profile_via_ctypes("/opt/axon/libaxon_pjrt.so")
    except Exception:
        mod._hook = None


# ---------------------------------------------------------------- wait split
def _split_waits(nc, max_waits=1):
    """walrus TPB_CTRL codegen rejects >1 sync-wait per instruction; move
    extras onto preceding NoOps on the same engine."""
    n_added = 0
    for fn in nc.m.functions:
        for blk in fn.blocks:
            new_insts = []
            for inst in blk.instructions:
                si = getattr(inst, "sync_info", None)
                waits = list(si.on_wait) if si is not None and si.on_wait else []
                if len(waits) > max_waits:
                    extra = waits[:-max_waits]
                    for i in range(0, len(extra), max_waits):
                        chunk = extra[i : i + max_waits]
                        nop = mybir.InstNoOp(
                            name=f"{inst.name}_wsplit{n_added}",
                            engine=inst.engine,
                            ins=[],
                            outs=[],
                            sync_info=mybir.SyncInfo(on_wait=chunk, on_update=[]),
                        )
                        n_added += 1
                        new_insts.append(nop)
                    si.on_wait = waits[-max_waits:]
                new_insts.append(inst)
            blk.instructions = new_insts
    return n_added


# ---------------------------------------------------------------- host prep
def _interleave16(vals):
    """[n] int16 -> [128, n/16] wrapped (i at [i%16, i//16]) and tiled x8."""
    n = vals.size
    a = np.zeros((16, n // 16), np.int16)
    a[np.arange(n) % 16, np.arange(n) // 16] = vals
    return np.tile(a, (8, 1))


def _prep(x, edge_index, batch, ncores):
    """Degree-balanced relabeling + blocked subrange chunk schedule."""
    N = x.shape[0]
    own = N // ncores
    gfull, rem = divmod(own, P)
    ngroups = gfull + (1 if rem else 0)

    src = np.concatenate([edge_index[0].astype(np.int64), np.arange(N)])
    dst = np.concatenate([edge_index[1].astype(np.int64), np.arange(N)])
    deg = np.bincount(dst, minlength=N)

    # degree-balanced bin packing (equal edge counts per group, all cores)
    caps = []
    for c in range(ncores):
        caps += [P] * gfull + ([rem] if rem else [])
    nbins = len(caps)
    import heapq

    heap = [(0, b) for b in range(nbins)]
    heapq.heapify(heap)
    bin_nodes = [[] for _ in range(nbins)]
    order = np.argsort(-deg, kind="stable")
    for node in order:
        while True:
            s, b = heapq.heappop(heap)
            if len(bin_nodes[b]) < caps[b]:
                break
        bin_nodes[b].append(node)
        if len(bin_nodes[b]) < caps[b]:
            heapq.heappush(heap, (s + int(deg[node]), b))

    perm = np.empty(N, np.int64)  # perm[new] = old
    for b in range(nbins):
        c, g = divmod(b, ngroups)
        base = c * own + g * P
        nodes = bin_nodes[b]
        perm[base : base + len(nodes)] = nodes
    inv = np.empty(N, np.int64)
    inv[perm] = np.arange(N)

    new_src = inv[src]
    new_dst = inv[dst]
    core_of = new_dst // own

    # per (core, group, subrange) edge lists and counts
    sub_of = new_src // SUBR
    n_cgr = np.zeros((ncores, ngroups, NSUB), np.int64)
    for c in range(ncores):
        m = core_of == c
        gg = (new_dst[m] % own) // P
        rr = sub_of[m]
        np.add.at(n_cgr[c], (gg, rr), 1)
    B = np.ceil(n_cgr.max(axis=0) / P).astype(np.int64)  # [ngroups, NSUB]
    S_list = B.sum(axis=1)  # chunks per group

    # block schedule (shared across cores)
    blocks = []
    tot_chunks = 0
    g0 = 0
    while g0 < ngroups:
        groups = list(range(g0, min(g0 + GBLK, ngroups)))
        chunk_group = []       # tile-chunk -> group
        chunk_sub = []         # tile-chunk -> subrange
        seg_start = {}         # (g, r) -> first tile-chunk (block-local)
        for r in range(NSUB):
            for g in groups:
                seg_start[(g, r)] = len(chunk_group)
                chunk_group += [g] * int(B[g, r])
                chunk_sub += [r] * int(B[g, r])
        CB = len(chunk_group)
        # gather calls: per r-region, sub-calls of <= CALL_CHUNKS chunks
        gcalls = []
        pos = 0
        for r in range(NSUB):
            nreg = int(B[groups, r].sum() if hasattr(B, "sum") else 0)
            nreg = int(sum(int(B[g, r]) for g in groups))
            q = 0
            while q < nreg:
                n = min(CALL_CHUNKS, nreg - q)
                gcalls.append((r, pos + q, n))
                q += n
            pos += nreg
        vcalls = []
        q = 0
        while q < CB:
            n = min(CALL_CHUNKS, CB - q)
            vcalls.append((q, n))
            q += n
        gfirst = {g: min(i for i, gg in enumerate(chunk_group) if gg == g)
                  for g in groups}
        glast = {g: max(i for i, gg in enumerate(chunk_group) if gg == g)
                 for g in groups}
        blocks.append(dict(groups=groups, CB=CB, chunk_group=chunk_group,
                           chunk_sub=chunk_sub, seg_start=seg_start,
                           gcalls=gcalls, vcalls=vcalls,
                           gfirst=gfirst, glast=glast,
                           chunk0=tot_chunks))
        tot_chunks += CB
        g0 += GBLK

    CBmax = max(b["CB"] for b in blocks)
    idxw = tot_chunks * 8  # int16 idx columns (128 rows per chunk / 16)

    # per-core static arrays
    per_core = []
    for c in range(ncores):
        m = core_of == c
        es = new_src[m]
        ed_own = new_dst[m] % own
        gg = ed_own // P
        rr = es // SUBR
        # order edges by (g, r) stable; k = running index within (g,r)
        o = np.lexsort((np.arange(es.size), rr, gg))
        es_s, ed_s, gg_s, rr_s = es[o], ed_own[o], gg[o], rr[o]
        key = gg_s * NSUB + rr_s
        firstk = np.r_[True, key[1:] != key[:-1]]
        startk = np.maximum.accumulate(np.where(firstk, np.arange(key.size), 0))
        k = np.arange(key.size) - startk

        srcidx_flat = np.zeros(tot_chunks * P, np.int16)
        dstidx_flat = np.zeros(tot_chunks * P, np.int16)
        dstloc = np.full((P, tot_chunks), 255.0, np.float32)

        # tile chunk for each edge
        blk_of_g = gg_s // GBLK
        seg0 = np.zeros(es_s.size, np.int64)
        chunk0 = np.zeros(es_s.size, np.int64)
        for bi, b in enumerate(blocks):
            mm = blk_of_g == bi
            if not mm.any():
                continue
            ss = np.array([b["seg_start"][(int(g), int(r))]
                           for g, r in zip(gg_s[mm], rr_s[mm])])
            seg0[mm] = ss + b["chunk0"]
        tchunk = seg0 + k // P
        p_e = k % P
        flat = tchunk * P + p_e
        srcidx_flat[flat] = (es_s - rr_s * SUBR).astype(np.int16)
        dstidx_flat[flat] = ed_s.astype(np.int16)
        dstloc.ravel()[p_e * tot_chunks + tchunk] = (ed_s - gg_s * P).astype(
            np.float32)

        # wrapped idx arrays per call (concatenated windows, 8 cols per chunk)
        srcidx16 = np.zeros((P, idxw), np.int16)
        dstidx16 = np.zeros((P, idxw), np.int16)
        for b in blocks:
            c0b = b["chunk0"]
            for (r, q, n) in b["gcalls"]:
                w0 = (c0b + q) * 8
                vals = srcidx_flat[(c0b + q) * P : (c0b + q + n) * P]
                srcidx16[:, w0 : w0 + n * 8] = _interleave16(vals)
            for (q, n) in b["vcalls"]:
                w0 = (c0b + q) * 8
                vals = dstidx_flat[(c0b + q) * P : (c0b + q + n) * P]
                dstidx16[:, w0 : w0 + n * 8] = _interleave16(vals)

        xT_own = np.ascontiguousarray(x[perm[c * own : (c + 1) * own]].T).astype(BF)

        bl = np.full((P, ngroups), 255.0, np.float32)
        for g in range(ngroups):
            size = P if (g < gfull or rem == 0) else rem
            ids = perm[c * own + g * P : c * own + g * P + size]
            bl[:size, g] = batch[ids]

        per_core.append(dict(
            srcidx16=srcidx16, dstidx16=dstidx16,
            dstloc=dstloc.astype(BF), xT_own=xT_own,
            batchloc=bl.astype(BF),
        ))

    meta = dict(N=N, own=own, ngroups=ngroups, gfull=gfull, rem=rem,
                ncores=ncores, blocks=blocks, tot_chunks=tot_chunks,
                CBmax=CBmax, idxw=idxw,
                S_list=[int(v) for v in S_list])
    return per_core, meta, perm


def _gsize(meta, g):
    return P if (g < meta["gfull"] or meta["rem"] == 0) else meta["rem"]


# ---------------------------------------------------------------- kernel build
def _build(meta, debug=False):
    own = meta["own"]
    ngroups = meta["ngroups"]
    ncores = meta["ncores"]
    blocks = meta["blocks"]
    tot_chunks = meta["tot_chunks"]
    CBmax = meta["CBmax"]
    idxw = meta["idxw"]

    nc = bass.Bass(target_bir_lowering=False, debug=True, num_swdge_queues=4)

    # ---- external inputs (per core)
    xT_in = nc.declare_dram_parameter("xT_own", [P, own], BF16, isOutput=False)
    srcidx_in = nc.declare_dram_parameter("srcidx16", [P, idxw], I16, isOutput=False)
    dstidx_in = nc.declare_dram_parameter("dstidx16", [P, idxw], I16, isOutput=False)
    dstloc_in = nc.declare_dram_parameter("dstloc", [P, tot_chunks], BF16,
                                          isOutput=False)
    batchloc_in = nc.declare_dram_parameter("batchloc", [P, ngroups], BF16,
                                            isOutput=False)
    wnames = [
        ("W1_l", [P, D]), ("W1_r", [P, D]), ("W2_l", [P, D]), ("W2_r", [P, D]),
        ("att1_rep", [P, D]), ("att2_rep", [P, D]),
        ("bias1_rep", [P, D]), ("bias2_rep", [P, D]),
        ("b1_l", [1, D]), ("b1_r", [1, D]), ("b2_l", [1, D]), ("b2_r", [1, D]),
        ("iota128", [P, P]), ("iota64", [P, NUM_GRAPHS]),
        ("W3", [P, 10]), ("b3row", [1, 10]), ("ones1", [1, P]),
        ("ones_col", [P, 1]),
    ]
    w_in = {n: nc.declare_dram_parameter(n, sh, BF16, isOutput=False)
            for n, sh in wnames}
    out_t = nc.declare_dram_parameter("out", [NUM_GRAPHS, 10], F32, isOutput=True)
    dbg = {}
    if debug:
        dbg["dbg_h1T"] = nc.declare_dram_parameter("dbg_h1T", [P, own], BF16,
                                                   isOutput=True)

    # ---- internal DRAM
    xl1_own = nc.dram_tensor("xl1_own", [own, D], BF16)
    xr1_own = nc.dram_tensor("xr1_own", [own, D], BF16)
    xl1_full = nc.dram_tensor("xl1_full", [own * ncores, D], BF16,
                              addr_space="Shared")
    xl2_own = nc.dram_tensor("xl2_own", [own, D], BF16)
    xr2_own = nc.dram_tensor("xr2_own", [own, D], BF16)
    xl2_full = nc.dram_tensor("xl2_full", [own * ncores, D], BF16,
                              addr_space="Shared")
    pool_stage = nc.dram_tensor("pool_stage", [NUM_GRAPHS, D + 1], F32)
    pool_red = nc.dram_tensor("pool_red", [NUM_GRAPHS, D + 1], F32,
                              addr_space="Shared")

    # ---- persistent SBUF
    xT_sb = nc.alloc_sbuf_tensor("xT_sb", [P, own], BF16)
    h1T_sb = nc.alloc_sbuf_tensor("h1T_sb", [P, own], BF16)
    dstloc_sb = nc.alloc_sbuf_tensor("dstloc_sb", [P, tot_chunks], BF16)
    batchloc_sb = nc.alloc_sbuf_tensor("batchloc_sb", [P, ngroups], BF16)
    ident_sb = nc.alloc_sbuf_tensor("ident_sb", [P, P], BF16)
    w_sb = {n: nc.alloc_sbuf_tensor(n + "_sb", sh, BF16) for n, sh in wnames}

    nreg = nc.gpsimd.alloc_register("nidx")

    def collective(kind, op, ins, outs):
        nc.gpsimd.collective_compute(
            kind, op, replica_groups=[list(range(ncores))], ins=ins, outs=outs
        )

    # ================= TC-load =================
    with TileContext(nc) as tc:
        with tc.tile_pool(name="idp", bufs=1) as idp:
            idt = idp.tile([P, P], F32)
            make_identity(nc, idt[:])
            nc.vector.tensor_copy(out=ident_sb[:], in_=idt[:])
        nc.gpsimd.load_library(library_config.mlp)
        nc.sync.dma_start(out=xT_sb[:], in_=xT_in[:])
        nc.sync.dma_start(out=dstloc_sb[:], in_=dstloc_in[:])
        nc.sync.dma_start(out=batchloc_sb[:], in_=batchloc_in[:])
        for n, _sh in wnames:
            nc.sync.dma_start(out=w_sb[n][:], in_=w_in[n][:])

    # ================= table build =================
    def build_tables(tc, srcT_sb, Wl, Wr, bl, br, out_l, out_r):
        with (
            tc.tile_pool(name="tp", bufs=3) as tp,
            tc.tile_pool(name="tpp", bufs=3, space="PSUM") as tpp,
        ):
            for g in range(ngroups):
                w = _gsize(meta, g)
                for W, brow, dest in ((Wl, bl, out_l), (Wr, br, out_r)):
                    ps = tpp.tile([P, D], F32, tag="ps")
                    nc.tensor.matmul(out=ps[:w, :],
                                     lhsT=srcT_sb[:, g * P : g * P + w],
                                     rhs=w_sb[W][:], start=True, stop=False)
                    nc.tensor.matmul(out=ps[:w, :], lhsT=w_sb["ones1"][:, :w],
                                     rhs=w_sb[brow][:], start=False, stop=True)
                    ot = tp.tile([P, D], BF16, tag="ot")
                    nc.scalar.activation(out=ot[:w, :], in_=ps[:w, :],
                                         func=mybir.ActivationFunctionType.Copy)
                    nc.sync.dma_start(out=dest[g * P : g * P + w, :],
                                      in_=ot[:w, :])

    # ================= edge layer =================
    def edge_layer(tc, xl_full_t, xr_own_t, att_rep, bias_rep, NH,
                   pool_ctx=None):
        C = D // NH
        Q = D + NH
        qn = [0]

        def next_q():
            qn[0] = (qn[0] + 1) % 4
            return qn[0]

        with (
            tc.tile_pool(name="gp", bufs=3) as gp,
            tc.tile_pool(name="vp", bufs=3) as vp,
            tc.tile_pool(name="rp", bufs=2) as rp,
            tc.tile_pool(name="ip", bufs=3) as ip,
            tc.tile_pool(name="indp", bufs=1) as indp,
            tc.tile_pool(name="sp", bufs=2) as sp,
            tc.tile_pool(name="ep", bufs=2) as ep,
            tc.tile_pool(name="aggp", bufs=2, space="PSUM") as aggp,
            tc.tile_pool(name="tpsum", bufs=2, space="PSUM") as tpsum,
        ):
            for b in blocks:
                CB = b["CB"]
                c0b = b["chunk0"]
                G = gp.tile([P, CBmax * D], BF16, tag="G")
                V = vp.tile([P, CBmax * D], BF16, tag="V")
                gidx = ip.tile([P, CBmax * 8], I16, tag="gidx")
                vidx = ip.tile([P, CBmax * 8], I16, tag="vidx")
                nc.sync.dma_start(out=gidx[:, : CB * 8],
                                  in_=srcidx_in[:, c0b * 8 : (c0b + CB) * 8])
                nc.sync.dma_start(out=vidx[:, : CB * 8],
                                  in_=dstidx_in[:, c0b * 8 : (c0b + CB) * 8])
                # gathers
                for (r, q, n) in b["gcalls"]:
                    nc.gpsimd.reg_mov(nreg, n * P)
                    nc.gpsimd.dma_gather(
                        G[:, q * D : (q + n) * D].rearrange(
                            "p (j c) -> p j c", c=D),
                        xl_full_t[r * SUBR : min((r + 1) * SUBR, own * ncores), :],
                        gidx[:, q * 8 : (q + n) * 8],
                        n * P, nreg, D, single_packet=False, queue_num=next_q())
                for (q, n) in b["vcalls"]:
                    nc.gpsimd.reg_mov(nreg, n * P)
                    nc.gpsimd.dma_gather(
                        V[:, q * D : (q + n) * D].rearrange(
                            "p (j c) -> p j c", c=D),
                        xr_own_t[:],
                        vidx[:, q * 8 : (q + n) * 8],
                        n * P, nreg, D, single_packet=False, queue_num=next_q())
                Gs = G[:, : CB * D]
                Vs = V[:, : CB * D]
                G3 = Gs.rearrange("p (m c) -> p m c", c=D)
                V3 = Vs.rearrange("p (m c) -> p m c", c=D)
                # z = G + V  (into V)
                nc.vector.tensor_tensor(out=V3, in0=G3, in1=V3,
                                        op=mybir.AluOpType.add)
                # leaky in place
                nc.vector.scalar_tensor_tensor(
                    out=Vs, in0=Vs, scalar=NEG_SLOPE, in1=Vs,
                    op0=mybir.AluOpType.mult, op1=mybir.AluOpType.max)
                # m = lr * att
                att_b = w_sb[att_rep][:].unsqueeze(1).broadcast_to([P, CB, D])
                nc.vector.tensor_tensor(out=V3, in0=V3, in1=att_b,
                                        op=mybir.AluOpType.mult)
                # score
                SC = sp.tile([P, CBmax * NH], F32, tag="SC")
                SCs = SC[:, : CB * NH]
                nc.vector.tensor_reduce(
                    out=SCs, in_=Vs.rearrange("p (mh c) -> p mh c", c=C),
                    axis=mybir.AxisListType.X, op=mybir.AluOpType.add)
                # p = exp(score): into PT (for p*G) and into RHS cols D..D+NH
                PT = sp.tile([P, CBmax * NH], BF16, tag="PT")
                PTs = PT[:, : CB * NH]
                nc.scalar.activation(out=PTs, in_=SCs,
                                     func=mybir.ActivationFunctionType.Exp)
                RHS = rp.tile([P, CBmax * Q], BF16, tag="RHS")
                RQ = RHS[:, : CB * Q].rearrange("p (t q) -> p t q", q=Q)
                nc.scalar.activation(
                    out=RQ[:, :, D : D + NH],
                    in_=SCs.rearrange("p (m h) -> p m h", h=NH),
                    func=mybir.ActivationFunctionType.Exp)
                # rhs = G * p
                p_b = PTs.rearrange("p (m h) -> p m h", h=NH).unsqueeze(
                    3).broadcast_to([P, CB, NH, C])
                nc.vector.tensor_tensor(
                    out=RQ[:, :, :D].rearrange("p t (h c) -> p t h c", h=NH),
                    in0=Gs.rearrange("p (m h c) -> p m h c", h=NH, c=C),
                    in1=p_b, op=mybir.AluOpType.mult)
                # indicator
                IND = indp.tile([P, CBmax * D], BF16, tag="IND")
                nc.vector.tensor_tensor(
                    out=IND[:, : CB * D].rearrange("p (t l) -> p t l", l=P),
                    in0=dstloc_sb[:, c0b : c0b + CB].unsqueeze(2).broadcast_to(
                        [P, CB, P]),
                    in1=w_sb["iota128"][:].unsqueeze(1).broadcast_to([P, CB, P]),
                    op=mybir.AluOpType.is_equal)
                # scatter matmuls + per-group epilogue
                aggs = {}
                for g in b["groups"]:
                    agt = aggp.tile([P, Q], F32, tag=f"agg{g % GBLK}",
                                    name=f"agg{g % GBLK}")
                    aggs[g] = agt
                for t in range(CB):
                    g = b["chunk_group"][t]
                    nc.tensor.matmul(
                        out=aggs[g][:],
                        lhsT=IND[:, t * D : (t + 1) * D],
                        rhs=RHS[:, t * Q : (t + 1) * Q],
                        start=(t == b["gfirst"][g]),
                        stop=(t == b["glast"][g]))
                for g in b["groups"]:
                    agg = aggs[g]
                    w = _gsize(meta, g)
                    DEN = ep.tile([P, NH], F32, tag="DEN")
                    nc.vector.tensor_scalar(out=DEN[:], in0=agg[:, D : D + NH],
                                            scalar1=1e-30, scalar2=None,
                                            op0=mybir.AluOpType.max)
                    REC = ep.tile([P, NH], F32, tag="REC")
                    nc.vector.reciprocal(out=REC[:], in_=DEN[:])
                    OUT = ep.tile([P, D], F32, tag="OUT")
                    rec_b = REC[:].unsqueeze(2).broadcast_to([P, NH, C])
                    nc.vector.tensor_tensor(
                        out=OUT[:].rearrange("p (h c) -> p h c", h=NH),
                        in0=agg[:, :D].rearrange("p (h c) -> p h c", h=NH),
                        in1=rec_b, op=mybir.AluOpType.mult)
                    nc.vector.tensor_tensor(out=OUT[:], in0=OUT[:],
                                            in1=w_sb[bias_rep][:],
                                            op=mybir.AluOpType.add)
                    NEG = ep.tile([P, D], F32, tag="NEG")
                    nc.vector.tensor_scalar(out=NEG[:], in0=OUT[:], scalar1=0.0,
                                            scalar2=None,
                                            op0=mybir.AluOpType.min)
                    EN = ep.tile([P, D], F32, tag="EN")
                    nc.scalar.activation(out=EN[:], in_=NEG[:],
                                         func=mybir.ActivationFunctionType.Exp)
                    nc.vector.tensor_scalar(out=OUT[:], in0=OUT[:], scalar1=0.0,
                                            scalar2=None,
                                            op0=mybir.AluOpType.max)
                    H = ep.tile([P, D], BF16, tag="H")
                    nc.vector.scalar_tensor_tensor(
                        out=H[:], in0=EN[:], scalar=-1.0, in1=OUT[:],
                        op0=mybir.AluOpType.add, op1=mybir.AluOpType.add)
                    if pool_ctx is None:
                        tps = tpsum.tile([P, P], BF16, tag="tps")
                        nc.tensor.transpose(out=tps[:, :w], in_=H[:w, :],
                                            identity=ident_sb[:w, :w])
                        nc.scalar.activation(
                            out=h1T_sb[:, g * P : g * P + w], in_=tps[:, :w],
                            func=mybir.ActivationFunctionType.Copy)
                    else:
                        (pool_psum,) = pool_ctx
                        pind = ep.tile([P, NUM_GRAPHS], BF16, tag="pind")
                        nc.vector.tensor_tensor(
                            out=pind[:],
                            in0=batchloc_sb[:, g : g + 1].to_broadcast(
                                [P, NUM_GRAPHS]),
                            in1=w_sb["iota64"][:],
                            op=mybir.AluOpType.is_equal)
                        prhs = ep.tile([P, D + 1], BF16, tag="prhs")
                        nc.vector.tensor_copy(out=prhs[:, :D], in_=H[:])
                        nc.vector.tensor_copy(out=prhs[:, D : D + 1],
                                              in_=w_sb["ones_col"][:])
                        nc.tensor.matmul(out=pool_psum[:], lhsT=pind[:],
                                         rhs=prhs[:],
                                         start=(g == 0),
                                         stop=(g == ngroups - 1))

    # ================= layer 1 =================
    with TileContext(nc) as tc:
        build_tables(tc, xT_sb, "W1_l", "W1_r", "b1_l", "b1_r",
                     xl1_own, xr1_own)

    with TileContext(nc) as tc:
        collective("AllGather", mybir.AluOpType.bypass, [xl1_own[:]],
                   [xl1_full[:]])
        edge_layer(tc, xl1_full, xr1_own, "att1_rep", "bias1_rep", 8)

    # ================= layer 2 =================
    with TileContext(nc) as tc:
        build_tables(tc, h1T_sb, "W2_l", "W2_r", "b2_l", "b2_r",
                     xl2_own, xr2_own)

    with TileContext(nc) as tc:
        collective("AllGather", mybir.AluOpType.bypass, [xl2_own[:]],
                   [xl2_full[:]])
        with tc.tile_pool(name="poolp", bufs=1, space="PSUM") as poolp, \
             tc.tile_pool(name="pstg", bufs=1) as pstg:
            pool_psum = poolp.tile([NUM_GRAPHS, D + 1], F32)
            edge_layer(tc, xl2_full, xr2_own, "att2_rep", "bias2_rep", 1,
                       pool_ctx=(pool_psum,))
            stg = pstg.tile([NUM_GRAPHS, D + 1], F32)
            nc.scalar.activation(out=stg[:], in_=pool_psum[:],
                                 func=mybir.ActivationFunctionType.Copy)
            nc.sync.dma_start(out=pool_stage[:], in_=stg[:])

    # ================= final head =================
    with TileContext(nc) as tc:
        collective("AllReduce", mybir.AluOpType.add, [pool_stage[:]],
                   [pool_red[:]])
        with (
            tc.tile_pool(name="fin", bufs=1) as fin,
            tc.tile_pool(name="finp", bufs=1, space="PSUM") as finp,
        ):
            red = fin.tile([NUM_GRAPHS, D + 1], F32)
            nc.sync.dma_start(out=red[:], in_=pool_red[:])
            cnt = fin.tile([NUM_GRAPHS, 1], F32)
            nc.vector.tensor_scalar(out=cnt[:], in0=red[:, D : D + 1],
                                    scalar1=1.0, scalar2=None,
                                    op0=mybir.AluOpType.max)
            rc = fin.tile([NUM_GRAPHS, 1], F32)
            nc.vector.reciprocal(out=rc[:], in_=cnt[:])
            pooled = fin.tile([NUM_GRAPHS, D], BF16)
            nc.vector.tensor_tensor(out=pooled[:], in0=red[:, :D],
                                    in1=rc[:].to_broadcast([NUM_GRAPHS, D]),
                                    op=mybir.AluOpType.mult)
            tp = finp.tile([P, NUM_GRAPHS], BF16)
            nc.tensor.transpose(out=tp[:], in_=pooled[:],
                                identity=ident_sb[:NUM_GRAPHS, :NUM_GRAPHS])
            pooledT = fin.tile([P, NUM_GRAPHS], BF16)
            nc.scalar.activation(out=pooledT[:], in_=tp[:],
                                 func=mybir.ActivationFunctionType.Copy)
            ops = finp.tile([NUM_GRAPHS, 10], F32)
            nc.tensor.matmul(out=ops[:], lhsT=pooledT[:], rhs=w_sb["W3"][:],
                             start=True, stop=False)
            nc.tensor.matmul(out=ops[:], lhsT=w_sb["ones1"][:, :NUM_GRAPHS],
                             rhs=w_sb["b3row"][:], start=False, stop=True)
            fout = fin.tile([NUM_GRAPHS, 10], F32)
            nc.scalar.activation(out=fout[:], in_=ops[:],
                                 func=mybir.ActivationFunctionType.Copy)
            nc.sync.dma_start(out=out_t[:], in_=fout[:])
        if debug:
            nc.sync.dma_start(out=dbg["dbg_h1T"][:], in_=h1T_sb[:])

    lower_extended_insts(nc)
    _split_waits(nc)
    return nc


# ---------------------------------------------------------------- entry point
def _run(x, edge_index, batch, W1_l, b1_l, W1_r, b1_r, att1, bias1,
         W2_l, b2_l, W2_r, b2_r, att2, bias2, W3, b3, ncores=NCORES,
         debug=False, trace=False):
    x = np.asarray(x, np.float32)
    per_core, meta, perm = _prep(np.asarray(x), np.asarray(edge_index),
                                 np.asarray(batch), ncores)
    consts = dict(
        W1_l=np.asarray(W1_l, np.float32).astype(BF),
        W1_r=np.asarray(W1_r, np.float32).astype(BF),
        W2_l=np.asarray(W2_l, np.float32).astype(BF),
        W2_r=np.asarray(W2_r, np.float32).astype(BF),
        att1_rep=np.tile(np.asarray(att1, np.float32).reshape(1, D),
                         (P, 1)).astype(BF),
        att2_rep=np.tile(np.asarray(att2, np.float32).reshape(1, D),
                         (P, 1)).astype(BF),
        bias1_rep=np.tile(np.asarray(bias1, np.float32).reshape(1, D),
                          (P, 1)).astype(BF),
        bias2_rep=np.tile(np.asarray(bias2, np.float32).reshape(1, D),
                          (P, 1)).astype(BF),
        b1_l=np.asarray(b1_l, np.float32).reshape(1, D).astype(BF),
        b1_r=np.asarray(b1_r, np.float32).reshape(1, D).astype(BF),
        b2_l=np.asarray(b2_l, np.float32).reshape(1, D).astype(BF),
        b2_r=np.asarray(b2_r, np.float32).reshape(1, D).astype(BF),
        iota128=np.tile(np.arange(P, dtype=np.float32).reshape(1, P),
                        (P, 1)).astype(BF),
        iota64=np.tile(np.arange(NUM_GRAPHS, dtype=np.float32).reshape(
            1, NUM_GRAPHS), (P, 1)).astype(BF),
        W3=np.asarray(W3, np.float32).astype(BF),
        b3row=np.asarray(b3, np.float32).reshape(1, 10).astype(BF),
        ones1=np.ones((1, P), np.float32).astype(BF),
        ones_col=np.ones((P, 1), np.float32).astype(BF),
    )
    nc = _build(meta, debug=debug)
    in_maps = []
    for c in range(ncores):
        m = dict(per_core[c])
        m.update(consts)
        in_maps.append(m)
    if trace:
        _install_profhook()
    res = run_bass_kernel_spmd(nc, in_maps, core_ids=list(range(ncores)),
                               trace=trace)
    return res.results[0]["out"].astype(np.float32), (res, per_core, meta, perm)


def kernel(**inputs):
    out, _res = _run(**inputs)
    return out



# revision 21
# speedup vs baseline: 1.5652x; 1.5652x over previous
"""2-layer GATv2 + global mean pool + linear head, on 8 Trainium2 NeuronCores.

Strategy v2 (dst-sharded, straddle-free packing, PE-based dst-feature bcast):
  - Nodes relabeled by degree-balanced bin-packing into groups of <=128
    destination nodes.  Core c owns nodes [c*own, (c+1)*own); edges assigned
    by destination owner.
  - Edges of a block (GBLK groups) are laid out per (block, src-subrange)
    REGION: groups packed consecutively, padding only at region end
    (B = ceil(max-core region size / 128) chunks of 128 edge slots).
  - One dma_gather per (block, region) fetches xl[src] (G) from the
    AllGather'd table (int16 idx windows of 25000 rows).
  - xr[dst] per edge (V) is NOT gathered: per chunk, V = INDT @ xr_group on
    the PE (INDT = fp8 one-hot [dst-local, edge-slot] loaded from DRAM);
    z = G + V on DVE, leaky on ACT (Prelu), att-dot + segment-softmax via
    per-chunk scatter matmuls with IND (fp8 one-hot, from DRAM) giving
    numerators and denominators in one PSUM accumulator per group.
  - Chunks may straddle group boundaries: such chunks simply get one
    IND/INDT matmul entry per touched group (host-scheduled, shared across
    cores; per-core all-zero indicator slices make inactive entries no-ops).
  - Layer tables xl/xr = x@W + b are built on device; xl is AllGather'ed,
    xr stays resident in SBUF.  Final pooling partials are AllReduce'd.
"""

import sys

for _p in ("/opt/trn_rl_repo",):
    if _p not in sys.path:
        sys.path.insert(0, _p)

import numpy as np
import ml_dtypes

BF = ml_dtypes.bfloat16
F8 = ml_dtypes.float8_e4m3

import concourse.bass as bass
import concourse.mybir as mybir
from concourse.tile import TileContext
from concourse.bass_utils import run_bass_kernel_spmd
from concourse.masks import make_identity
from concourse import library_config
from concourse.library_overlay import lower_extended_insts

F32 = mybir.dt.float32
BF16 = mybir.dt.bfloat16
FP8 = mybir.dt.float8e4
I16 = mybir.dt.int16
P = 128
D = 128
NCORES = 8
NUM_GRAPHS = 64
NEG_SLOPE = 0.2
SUBR = 25000          # int16 dma_gather window (4 x 25000 = 100000 rows)
NSUB = 4
GBLK = 2              # groups per processing block
ZB = 4                # chunks per PSUM z tile (4*128 f32 = 2KB/partition)


# ---------------------------------------------------------------- prof hook
def _install_profhook():
    """Provide antenv.axon_hooks (absent in this image) so trace=True works."""
    import types

    if "antenv.axon_hooks" in sys.modules:
        return
    try:
        from trn_agent_boot.trn_boot import _ntff_profile_via_ctypes
    except Exception:
        return
    mod = types.ModuleType("antenv.axon_hooks")
    mod._hook = None
    mod.set_axon_ntff_profile_hook = lambda h: setattr(mod, "_hook", h)
    mod.get_axon_ntff_profile_hook = lambda: mod._hook
    sys.modules["antenv.axon_hooks"] = mod
    try:
        mod._hook = _ntff_profile_via_ctypes("/opt/axon/libaxon_pjrt.so")
    except Exception:
        mod._hook = None


# ---------------------------------------------------------------- wait split
def _split_waits(nc, max_waits=1):
    """walrus TPB_CTRL codegen rejects >1 sync-wait per instruction; move
    extras onto preceding NoOps on the same engine."""
    n_added = 0
    for fn in nc.m.functions:
        for blk in fn.blocks:
            new_insts = []
            for inst in blk.instructions:
                si = getattr(inst, "sync_info", None)
                waits = list(si.on_wait) if si is not None and si.on_wait else []
                if len(waits) > max_waits:
                    extra = waits[:-max_waits]
                    for i in range(0, len(extra), max_waits):
                        chunk = extra[i : i + max_waits]
                        nop = mybir.InstNoOp(
                            name=f"{inst.name}_wsplit{n_added}",
                            engine=inst.engine,
                            ins=[],
                            outs=[],
                            sync_info=mybir.SyncInfo(on_wait=chunk, on_update=[]),
                        )
                        n_added += 1
                        new_insts.append(nop)
                    si.on_wait = waits[-max_waits:]
                new_insts.append(inst)
            blk.instructions = new_insts
    return n_added


# ---------------------------------------------------------------- host prep
def _interleave16(vals):
    """[n] int16 -> [128, n/16] wrapped (i at [i%16, i//16]) and tiled x8."""
    n = vals.size
    a = np.zeros((16, n // 16), np.int16)
    a[np.arange(n) % 16, np.arange(n) // 16] = vals
    return np.tile(a, (8, 1))


def _prep(x, edge_index, batch, ncores):
    """Degree-balanced relabeling + straddle-free region schedule."""
    N = x.shape[0]
    own = N // ncores
    gfull, rem = divmod(own, P)
    ngroups = gfull + (1 if rem else 0)
    nblocks = (ngroups + GBLK - 1) // GBLK

    # self-loops are handled as per-block "self chunks" (no gather), so the
    # edge stream here is only the real edges
    src = edge_index[0].astype(np.int64)
    dst = edge_index[1].astype(np.int64)
    deg = np.bincount(dst, minlength=N) + 1  # +1: self-loop balance weight

    # degree-balanced bin packing (equal edge counts per group, all cores)
    caps = []
    for c in range(ncores):
        caps += [P] * gfull + ([rem] if rem else [])
    nbins = len(caps)
    import heapq

    heap = [(0, b) for b in range(nbins)]
    heapq.heapify(heap)
    bin_nodes = [[] for _ in range(nbins)]
    order = np.argsort(-deg, kind="stable")
    for node in order:
        while True:
            s, b = heapq.heappop(heap)
            if len(bin_nodes[b]) < caps[b]:
                break
        bin_nodes[b].append(node)
        if len(bin_nodes[b]) < caps[b]:
            heapq.heappush(heap, (s + int(deg[node]), b))

    perm = np.empty(N, np.int64)  # perm[new] = old
    for b in range(nbins):
        c, g = divmod(b, ngroups)
        base = c * own + g * P
        nodes = bin_nodes[b]
        perm[base : base + len(nodes)] = nodes
    inv = np.empty(N, np.int64)
    inv[perm] = np.arange(N)

    new_src = inv[src]
    new_dst = inv[dst]
    core_of = new_dst // own

    # per-core edge keys
    # region = (block, subrange); within region, group-j edges consecutive
    sub_of = new_src // SUBR

    # per (core, block, r, j) counts
    cnt = np.zeros((ncores, nblocks, NSUB, GBLK), np.int64)
    ed_data = []
    for c in range(ncores):
        m = core_of == c
        es = new_src[m]
        ed_own = new_dst[m] % own
        gg = ed_own // P
        bb = gg // GBLK
        jj = gg % GBLK
        rr = es // SUBR
        np.add.at(cnt[c], (bb, rr, jj), 1)
        o = np.lexsort((jj, rr, bb))
        ed_data.append((es[o], ed_own[o], bb[o], rr[o], jj[o]))

    E_cbr = cnt.sum(axis=3)                      # [ncores, nblocks, NSUB]
    Emax = E_cbr.max(axis=0)                     # [nblocks, NSUB]
    B = np.ceil(Emax / P).astype(np.int64)       # chunks per region

    # shared schedule
    blocks = []
    tot_chunks = 0
    nmm_tot = 0
    for b in range(nblocks):
        groups = list(range(b * GBLK, min((b + 1) * GBLK, ngroups)))
        njb = len(groups)
        chunk0 = tot_chunks
        regs = []        # (r, B_r, chunk_off_in_block)
        coff = 0
        for r in range(NSUB):
            regs.append((r, int(B[b, r]), coff))
            coff += int(B[b, r])
        CBg = coff                  # gathered chunks
        CB = CBg + njb              # + one self chunk per group
        # mm entries: (tloc, j) active if any core has group-j edges in chunk
        entries = []
        for (r, Br, co) in regs:
            for t in range(Br):
                lo, hi = t * P, (t + 1) * P
                for j in range(njb):
                    act = False
                    for c in range(ncores):
                        s0 = int(cnt[c, b, r, :j].sum())
                        s1 = s0 + int(cnt[c, b, r, j])
                        if max(lo, s0) < min(hi, s1):
                            act = True
                            break
                    if act:
                        entries.append((co + t, j))
        for j in range(njb):        # self chunks (identity indicators)
            entries.append((CBg + j, j))
        # start/stop per j over the block's entry list
        first_j = {}
        last_j = {}
        for ei, (t, j) in enumerate(entries):
            if j not in first_j:
                first_j[j] = ei
            last_j[j] = ei
        # per-chunk entry spans (for V matmul start/stop)
        ch_first = {}
        ch_last = {}
        for ei, (t, j) in enumerate(entries):
            if t not in ch_first:
                ch_first[t] = ei
            ch_last[t] = ei
        assert all(t in ch_first for t in range(CB)), "empty chunk"
        blocks.append(dict(b=b, groups=groups, njb=njb, CB=CB, CBg=CBg,
                           regs=regs,
                           entries=entries, first_j=first_j, last_j=last_j,
                           ch_first=ch_first, ch_last=ch_last,
                           chunk0=chunk0, mm0=nmm_tot))
        tot_chunks += CB
        nmm_tot += len(entries)

    CBmax = max(bl["CB"] for bl in blocks)
    NMMmax = max(len(bl["entries"]) for bl in blocks)
    idxw = tot_chunks * 8

    # per-core static arrays
    per_core = []
    for c in range(ncores):
        es, ed_own, bb, rr, jj = ed_data[c]
        # slot within region: edges of (b, r) are consecutive (sorted by j)
        key = bb * NSUB + rr
        firstk = np.r_[True, key[1:] != key[:-1]]
        startk = np.maximum.accumulate(np.where(firstk, np.arange(key.size), 0))
        slot = np.arange(key.size) - startk

        # global chunk + in-chunk position
        reg_chunk0 = np.zeros((nblocks, NSUB), np.int64)
        for bl in blocks:
            for (r, Br, co) in bl["regs"]:
                reg_chunk0[bl["b"], r] = bl["chunk0"] + co
        tglob = reg_chunk0[bb, rr] + slot // P
        pp = slot % P

        # srcidx (wrapped per region call; pads -> 0)
        srcflat = np.zeros(tot_chunks * P, np.int16)
        srcflat[tglob * P + pp] = (es - rr * SUBR).astype(np.int16)
        srcidx16 = np.zeros((P, idxw), np.int16)
        for bl in blocks:
            for (r, Br, co) in bl["regs"]:
                c0 = bl["chunk0"] + co
                vals = srcflat[c0 * P : (c0 + Br) * P]
                srcidx16[:, c0 * 8 : (c0 + Br) * 8] = _interleave16(vals)

        # entry id per (tglob, j)
        entmap = np.full((tot_chunks, GBLK), -1, np.int64)
        for bl in blocks:
            for ei, (t, j) in enumerate(bl["entries"]):
                entmap[bl["chunk0"] + t, j] = bl["mm0"] + ei
        eid = entmap[tglob, jj]
        assert (eid >= 0).all()
        dloc = ed_own % P   # dst local within group

        IND8 = np.zeros((P, nmm_tot * P), F8)
        INDT8 = np.zeros((P, nmm_tot * P), F8)
        IND8[pp, eid * P + dloc] = 1.0
        INDT8[dloc, eid * P + pp] = 1.0
        # self-chunk identity entries (first w diag elements per group)
        for bl in blocks:
            for j, g in enumerate(bl["groups"]):
                w = P if (g < gfull or rem == 0) else rem
                se = entmap[bl["chunk0"] + bl["CBg"] + j, j]
                diag = np.arange(w)
                IND8[diag, se * P + diag] = 1.0
                INDT8[diag, se * P + diag] = 1.0

        xT_own = np.ascontiguousarray(x[perm[c * own : (c + 1) * own]].T).astype(BF)

        bl_arr = np.full((P, ngroups), 255.0, np.float32)
        for g in range(ngroups):
            size = P if (g < gfull or rem == 0) else rem
            ids = perm[c * own + g * P : c * own + g * P + size]
            bl_arr[:size, g] = batch[ids]

        per_core.append(dict(
            srcidx16=srcidx16, IND8=IND8, INDT8=INDT8, xT_own=xT_own,
            batchloc=bl_arr.astype(BF),
        ))

    meta = dict(N=N, own=own, ngroups=ngroups, gfull=gfull, rem=rem,
                ncores=ncores, blocks=blocks, tot_chunks=tot_chunks,
                nmm_tot=nmm_tot, CBmax=CBmax, NMMmax=NMMmax, idxw=idxw)
    return per_core, meta, perm


def _gsize(meta, g):
    return P if (g < meta["gfull"] or meta["rem"] == 0) else meta["rem"]


# ---------------------------------------------------------------- kernel build
def _build(meta, debug=False):
    own = meta["own"]
    ngroups = meta["ngroups"]
    ncores = meta["ncores"]
    blocks = meta["blocks"]
    tot_chunks = meta["tot_chunks"]
    nmm_tot = meta["nmm_tot"]
    CBmax = meta["CBmax"]
    NMMmax = meta["NMMmax"]
    idxw = meta["idxw"]

    nc = bass.Bass(target_bir_lowering=False, debug=True, num_swdge_queues=4)

    # ---- external inputs (per core)
    xT_in = nc.declare_dram_parameter("xT_own", [P, own], BF16, isOutput=False)
    srcidx_in = nc.declare_dram_parameter("srcidx16", [P, idxw], I16, isOutput=False)
    ind_in = nc.declare_dram_parameter("IND8", [P, nmm_tot * P], FP8, isOutput=False)
    indt_in = nc.declare_dram_parameter("INDT8", [P, nmm_tot * P], FP8,
                                        isOutput=False)
    batchloc_in = nc.declare_dram_parameter("batchloc", [P, ngroups], BF16,
                                            isOutput=False)
    wnames = [
        ("W1_l", [P, D]), ("W1_r", [P, D]), ("W2_l", [P, D]), ("W2_r", [P, D]),
        ("att1_rep", [P, D]), ("att2_rep", [P, D]),
        ("bias1_rep", [P, D]), ("bias2_rep", [P, D]),
        ("b1_l", [1, D]), ("b1_r", [1, D]), ("b2_l", [1, D]), ("b2_r", [1, D]),
        ("iota64", [P, NUM_GRAPHS]),
        ("W3", [P, 10]), ("b3row", [1, 10]), ("ones1", [1, P]),
        ("ones_col", [P, 1]),
    ]
    w_in = {n: nc.declare_dram_parameter(n, sh, BF16, isOutput=False)
            for n, sh in wnames}
    out_t = nc.declare_dram_parameter("out", [NUM_GRAPHS, 10], F32, isOutput=True)
    dbg = {}
    if debug:
        dbg["dbg_h1T"] = nc.declare_dram_parameter("dbg_h1T", [P, own], BF16,
                                                   isOutput=True)

    # ---- internal DRAM
    xl1_own = nc.dram_tensor("xl1_own", [own, D], BF16)
    xl1_full = nc.dram_tensor("xl1_full", [own * ncores, D], BF16,
                              addr_space="Shared")
    xl2_own = nc.dram_tensor("xl2_own", [own, D], BF16)
    xl2_full = nc.dram_tensor("xl2_full", [own * ncores, D], BF16,
                              addr_space="Shared")
    pool_stage = nc.dram_tensor("pool_stage", [NUM_GRAPHS, D + 1], F32)
    pool_red = nc.dram_tensor("pool_red", [NUM_GRAPHS, D + 1], F32,
                              addr_space="Shared")

    # ---- persistent SBUF
    h1T_sb = nc.alloc_sbuf_tensor("h1T_sb", [P, own], BF16)
    xr_sb = nc.alloc_sbuf_tensor("xr_sb", [P, ngroups * D], BF16)
    xlo_sb = nc.alloc_sbuf_tensor("xlo_sb", [P, ngroups * D], BF16)
    batchloc_sb = nc.alloc_sbuf_tensor("batchloc_sb", [P, ngroups], BF16)
    ident_sb = nc.alloc_sbuf_tensor("ident_sb", [P, P], BF16)
    w_sb = {n: nc.alloc_sbuf_tensor(n + "_sb", sh, BF16) for n, sh in wnames}

    nreg = nc.gpsimd.alloc_register("nidx")

    def collective(kind, op, ins, outs):
        nc.gpsimd.collective_compute(
            kind, op, replica_groups=[list(range(ncores))], ins=ins, outs=outs
        )

    # ================= TC-load =================
    with TileContext(nc) as tc:
        with tc.tile_pool(name="idp", bufs=1) as idp:
            idt = idp.tile([P, P], F32)
            make_identity(nc, idt[:])
            nc.vector.tensor_copy(out=ident_sb[:], in_=idt[:])
        nc.gpsimd.load_library(library_config.mlp)
        nc.sync.dma_start(out=batchloc_sb[:], in_=batchloc_in[:])
        for n, _sh in wnames:
            nc.sync.dma_start(out=w_sb[n][:], in_=w_in[n][:])

    # ================= table build =================
    def build_tables(tc, srcT_dram, srcT_sbuf, Wl, Wr, bl, br, out_l):
        """xl -> SBUF xlo_sb (+ DRAM out_l); xr -> SBUF xr_sb.
        Source rows come from srcT_dram (streamed) or srcT_sbuf."""
        with (
            tc.tile_pool(name="tp", bufs=3) as tp,
            tc.tile_pool(name="tpp", bufs=4, space="PSUM") as tpp,
        ):
            for g in range(ngroups):
                w = _gsize(meta, g)
                if srcT_dram is not None:
                    st = tp.tile([P, P], BF16, tag="st")
                    nc.sync.dma_start(out=st[:, :w],
                                      in_=srcT_dram[:, g * P : g * P + w])
                    lhs = st[:, :w]
                else:
                    lhs = srcT_sbuf[:, g * P : g * P + w]
                # xl
                if w < P:
                    nc.vector.memset(xlo_sb[:, g * D : g * D + D], 0.0)
                    nc.vector.memset(xr_sb[:, g * D : g * D + D], 0.0)
                ps = tpp.tile([P, D], F32, tag="ps")
                nc.tensor.matmul(out=ps[:w, :], lhsT=lhs,
                                 rhs=w_sb[Wl][:], start=True, stop=False)
                nc.tensor.matmul(out=ps[:w, :], lhsT=w_sb["ones1"][:, :w],
                                 rhs=w_sb[bl][:], start=False, stop=True)
                nc.scalar.activation(out=xlo_sb[:w, g * D : g * D + D],
                                     in_=ps[:w, :],
                                     func=mybir.ActivationFunctionType.Copy)
                nc.sync.dma_start(out=out_l[g * P : g * P + w, :],
                                  in_=xlo_sb[:w, g * D : g * D + D])
                # xr -> SBUF
                ps2 = tpp.tile([P, D], F32, tag="ps2")
                nc.tensor.matmul(out=ps2[:w, :], lhsT=lhs,
                                 rhs=w_sb[Wr][:], start=True, stop=False)
                nc.tensor.matmul(out=ps2[:w, :], lhsT=w_sb["ones1"][:, :w],
                                 rhs=w_sb[br][:], start=False, stop=True)
                nc.scalar.activation(out=xr_sb[:w, g * D : g * D + D],
                                     in_=ps2[:w, :],
                                     func=mybir.ActivationFunctionType.Copy)

    # ================= edge layer =================
    def edge_layer(tc, xl_full_t, att_rep, bias_rep, NH, pool_ctx=None):
        C = D // NH
        Q = D + NH
        qn = [0]

        def next_q():
            qn[0] = (qn[0] + 1) % 4
            return qn[0]

        with (
            tc.tile_pool(name="gp", bufs=2) as gp,
            tc.tile_pool(name="zp", bufs=2) as zp,
            tc.tile_pool(name="rp", bufs=2) as rp,
            tc.tile_pool(name="ip", bufs=2) as ip,
            tc.tile_pool(name="sp", bufs=2) as sp,
            tc.tile_pool(name="ep", bufs=2) as ep,
            tc.tile_pool(name="zpsum", bufs=2, space="PSUM") as zpsum,
            tc.tile_pool(name="aggp", bufs=2, space="PSUM") as aggp,
            tc.tile_pool(name="tpsum", bufs=1, space="PSUM") as tpsum,
        ):
            for bl in blocks:
                CB = bl["CB"]
                c0b = bl["chunk0"]
                mm0 = bl["mm0"]
                nmm_b = len(bl["entries"])
                njb = bl["njb"]
                G = gp.tile([P, CBmax * D], BF16, tag="G")
                Z = zp.tile([P, CBmax * D], BF16, tag="Z")
                IND = ip.tile([P, NMMmax * P], FP8, tag="IND")
                INDT = ip.tile([P, NMMmax * P], FP8, tag="INDT")
                sidx = ip.tile([P, CBmax * 8], I16, tag="sidx")
                nc.sync.dma_start(out=sidx[:, : CB * 8],
                                  in_=srcidx_in[:, c0b * 8 : (c0b + CB) * 8])
                nc.sync.dma_start(
                    out=IND[:, : nmm_b * P],
                    in_=ind_in[:, mm0 * P : (mm0 + nmm_b) * P])
                nc.sync.dma_start(
                    out=INDT[:, : nmm_b * P],
                    in_=indt_in[:, mm0 * P : (mm0 + nmm_b) * P])
                # gathers: one per region
                for (r, Br, co) in bl["regs"]:
                    if Br == 0:
                        continue
                    n_idx = Br * P
                    nc.gpsimd.reg_mov(nreg, n_idx)
                    nc.gpsimd.dma_gather(
                        G[:, co * D : (co + Br) * D].rearrange(
                            "p (j c) -> p j c", c=D),
                        xl_full_t[r * SUBR : (r + 1) * SUBR, :],
                        sidx[:, co * 8 : (co + Br) * 8],
                        n_idx, nreg, D, single_packet=False, queue_num=next_q())
                # self chunks: G rows = own xl rows
                CBg = bl["CBg"]
                for j, g in enumerate(bl["groups"]):
                    nc.vector.tensor_copy(
                        out=G[:, (CBg + j) * D : (CBg + j + 1) * D],
                        in_=xlo_sb[:, g * D : (g + 1) * D])

                # V matmuls + z-add + leaky, in ZB-chunk slices
                ents = bl["entries"]
                for z0 in range(0, CB, ZB):
                    zn = min(ZB, CB - z0)
                    zps = zpsum.tile([P, ZB * D], F32, tag="zps")
                    for t in range(z0, z0 + zn):
                        e0, e1 = bl["ch_first"][t], bl["ch_last"][t]
                        for ei in range(e0, e1 + 1):
                            tt, j = ents[ei]
                            assert tt == t
                            g = bl["groups"][j]
                            nc.tensor.matmul(
                                out=zps[:, (t - z0) * D : (t - z0 + 1) * D],
                                lhsT=INDT[:, ei * P : (ei + 1) * P],
                                rhs=xr_sb[:, g * D : (g + 1) * D],
                                start=(ei == e0), stop=(ei == e1))
                    # z = G + V
                    nc.vector.tensor_tensor(
                        out=Z[:, z0 * D : (z0 + zn) * D],
                        in0=G[:, z0 * D : (z0 + zn) * D],
                        in1=zps[:, : zn * D],
                        op=mybir.AluOpType.add)
                    # leaky in place (ACT)
                    nc.scalar.activation(
                        out=Z[:, z0 * D : (z0 + zn) * D],
                        in_=Z[:, z0 * D : (z0 + zn) * D],
                        func=mybir.ActivationFunctionType.Prelu,
                        alpha=NEG_SLOPE)
                Zs = Z[:, : CB * D]
                Z3 = Zs.rearrange("p (m c) -> p m c", c=D)
                # m = lr * att
                att_b = w_sb[att_rep][:].unsqueeze(1).broadcast_to([P, CB, D])
                nc.vector.tensor_tensor(out=Z3, in0=Z3, in1=att_b,
                                        op=mybir.AluOpType.mult)
                # score
                SC = sp.tile([P, CBmax * NH], F32, tag="SC")
                SCs = SC[:, : CB * NH]
                nc.vector.tensor_reduce(
                    out=SCs, in_=Zs.rearrange("p (mh c) -> p mh c", c=C),
                    axis=mybir.AxisListType.X, op=mybir.AluOpType.add)
                # p = exp(score): into PT (for p*G) and into RHS cols D..D+NH
                PT = sp.tile([P, CBmax * NH], BF16, tag="PT")
                PTs = PT[:, : CB * NH]
                nc.scalar.activation(out=PTs, in_=SCs,
                                     func=mybir.ActivationFunctionType.Exp)
                RHS = rp.tile([P, CBmax * Q], BF16, tag="RHS")
                RQ = RHS[:, : CB * Q].rearrange("p (t q) -> p t q", q=Q)
                nc.scalar.activation(
                    out=RQ[:, :, D : D + NH],
                    in_=SCs.rearrange("p (m h) -> p m h", h=NH),
                    func=mybir.ActivationFunctionType.Exp)
                # rhs = G * p
                p_b = PTs.rearrange("p (m h) -> p m h", h=NH).unsqueeze(
                    3).broadcast_to([P, CB, NH, C])
                nc.vector.tensor_tensor(
                    out=RQ[:, :, :D].rearrange("p t (h c) -> p t h c", h=NH),
                    in0=G[:, : CB * D].rearrange("p (m h c) -> p m h c",
                                                 h=NH, c=C),
                    in1=p_b, op=mybir.AluOpType.mult)
                # scatter matmuls into per-group agg tiles (separate PSUM
                # banks: interleaved accumulation chains must not share one)
                aggs = [aggp.tile([P, Q], F32, tag=f"agg{j}",
                                  name=f"agg{j}") for j in range(njb)]
                for ei, (t, j) in enumerate(ents):
                    nc.tensor.matmul(
                        out=aggs[j][:],
                        lhsT=IND[:, ei * P : (ei + 1) * P],
                        rhs=RHS[:, t * Q : (t + 1) * Q],
                        start=(ei == bl["first_j"][j]),
                        stop=(ei == bl["last_j"][j]))
                # ---- epilogue: copy aggs to one SBUF tile, then batched ops
                AGS = ep.tile([P, GBLK * Q], F32, tag="AGS")
                for j in range(njb):
                    nc.scalar.activation(
                        out=AGS[:, j * Q : (j + 1) * Q], in_=aggs[j][:],
                        func=mybir.ActivationFunctionType.Copy)
                a3 = AGS[:, : njb * Q].rearrange("p (j q) -> p j q", q=Q)
                DEN = ep.tile([P, GBLK * NH], F32, tag="DEN")
                DENs = DEN[:, : njb * NH].rearrange("p (j h) -> p j h", h=NH)
                nc.vector.tensor_scalar(out=DENs, in0=a3[:, :, D : D + NH],
                                        scalar1=1e-30, scalar2=None,
                                        op0=mybir.AluOpType.max)
                REC = ep.tile([P, GBLK * NH], F32, tag="REC")
                nc.vector.reciprocal(out=REC[:, : njb * NH],
                                     in_=DEN[:, : njb * NH])
                OUT = ep.tile([P, GBLK * D], F32, tag="OUT")
                rec_b = REC[:, : njb * NH].rearrange(
                    "p (j h) -> p j h", h=NH).unsqueeze(3).broadcast_to(
                    [P, njb, NH, C])
                nc.vector.tensor_tensor(
                    out=OUT[:, : njb * D].rearrange(
                        "p (j h c) -> p j h c", h=NH, c=C),
                    in0=a3[:, :, :D].rearrange(
                        "p j (h c) -> p j h c", h=NH),
                    in1=rec_b, op=mybir.AluOpType.mult)
                bias_b = w_sb[bias_rep][:].unsqueeze(1).broadcast_to(
                    [P, njb, D])
                nc.vector.tensor_tensor(
                    out=OUT[:, : njb * D].rearrange("p (j c) -> p j c", c=D),
                    in0=OUT[:, : njb * D].rearrange("p (j c) -> p j c", c=D),
                    in1=bias_b, op=mybir.AluOpType.add)
                NEG = ep.tile([P, GBLK * D], F32, tag="NEG")
                nc.vector.tensor_scalar(out=NEG[:, : njb * D],
                                        in0=OUT[:, : njb * D], scalar1=0.0,
                                        scalar2=None,
                                        op0=mybir.AluOpType.min)
                EN = ep.tile([P, GBLK * D], F32, tag="EN")
                nc.scalar.activation(out=EN[:, : njb * D],
                                     in_=NEG[:, : njb * D],
                                     func=mybir.ActivationFunctionType.Exp)
                nc.vector.tensor_scalar(out=OUT[:, : njb * D],
                                        in0=OUT[:, : njb * D], scalar1=0.0,
                                        scalar2=None,
                                        op0=mybir.AluOpType.max)
                H = ep.tile([P, GBLK * D], BF16, tag="H")
                nc.vector.scalar_tensor_tensor(
                    out=H[:, : njb * D], in0=EN[:, : njb * D], scalar=-1.0,
                    in1=OUT[:, : njb * D],
                    op0=mybir.AluOpType.add, op1=mybir.AluOpType.add)
                for j, g in enumerate(bl["groups"]):
                    w = _gsize(meta, g)
                    Hj = H[:, j * D : (j + 1) * D]
                    if pool_ctx is None:
                        tps = tpsum.tile([P, P], BF16, tag="tps")
                        nc.tensor.transpose(out=tps[:, :w], in_=Hj[:w, :],
                                            identity=ident_sb[:w, :w])
                        nc.scalar.activation(
                            out=h1T_sb[:, g * P : g * P + w], in_=tps[:, :w],
                            func=mybir.ActivationFunctionType.Copy)
                    else:
                        (pool_psum,) = pool_ctx
                        pind = ep.tile([P, NUM_GRAPHS], BF16, tag="pind")
                        nc.vector.tensor_tensor(
                            out=pind[:],
                            in0=batchloc_sb[:, g : g + 1].to_broadcast(
                                [P, NUM_GRAPHS]),
                            in1=w_sb["iota64"][:],
                            op=mybir.AluOpType.is_equal)
                        prhs = ep.tile([P, D + 1], BF16, tag="prhs")
                        nc.vector.tensor_copy(out=prhs[:, :D], in_=Hj)
                        nc.vector.tensor_copy(out=prhs[:, D : D + 1],
                                              in_=w_sb["ones_col"][:])
                        nc.tensor.matmul(out=pool_psum[:], lhsT=pind[:],
                                         rhs=prhs[:],
                                         start=(g == 0),
                                         stop=(g == ngroups - 1))

    # ================= layer 1 =================
    with TileContext(nc) as tc:
        build_tables(tc, xT_in, None, "W1_l", "W1_r", "b1_l", "b1_r", xl1_own)

    with TileContext(nc) as tc:
        collective("AllGather", mybir.AluOpType.bypass, [xl1_own[:]],
                   [xl1_full[:]])
        edge_layer(tc, xl1_full, "att1_rep", "bias1_rep", 8)

    # ================= layer 2 =================
    with TileContext(nc) as tc:
        build_tables(tc, None, h1T_sb, "W2_l", "W2_r", "b2_l", "b2_r", xl2_own)

    with TileContext(nc) as tc:
        collective("AllGather", mybir.AluOpType.bypass, [xl2_own[:]],
                   [xl2_full[:]])
        with tc.tile_pool(name="poolp", bufs=1, space="PSUM") as poolp, \
             tc.tile_pool(name="pstg", bufs=1) as pstg:
            pool_psum = poolp.tile([NUM_GRAPHS, D + 1], F32)
            edge_layer(tc, xl2_full, "att2_rep", "bias2_rep", 1,
                       pool_ctx=(pool_psum,))
            stg = pstg.tile([NUM_GRAPHS, D + 1], F32)
            nc.scalar.activation(out=stg[:], in_=pool_psum[:],
                                 func=mybir.ActivationFunctionType.Copy)
            nc.sync.dma_start(out=pool_stage[:], in_=stg[:])

    # ================= final head =================
    with TileContext(nc) as tc:
        collective("AllReduce", mybir.AluOpType.add, [pool_stage[:]],
                   [pool_red[:]])
        with (
            tc.tile_pool(name="fin", bufs=1) as fin,
            tc.tile_pool(name="finp", bufs=1, space="PSUM") as finp,
        ):
            red = fin.tile([NUM_GRAPHS, D + 1], F32)
            nc.sync.dma_start(out=red[:], in_=pool_red[:])
            cnt = fin.tile([NUM_GRAPHS, 1], F32)
            nc.vector.tensor_scalar(out=cnt[:], in0=red[:, D : D + 1],
                                    scalar1=1.0, scalar2=None,
                                    op0=mybir.AluOpType.max)
            rc = fin.tile([NUM_GRAPHS, 1], F32)
            nc.vector.reciprocal(out=rc[:], in_=cnt[:])
            pooled = fin.tile([NUM_GRAPHS, D], BF16)
            nc.vector.tensor_tensor(out=pooled[:], in0=red[:, :D],
                                    in1=rc[:].to_broadcast([NUM_GRAPHS, D]),
                                    op=mybir.AluOpType.mult)
            tp = finp.tile([P, NUM_GRAPHS], BF16)
            nc.tensor.transpose(out=tp[:], in_=pooled[:],
                                identity=ident_sb[:NUM_GRAPHS, :NUM_GRAPHS])
            pooledT = fin.tile([P, NUM_GRAPHS], BF16)
            nc.scalar.activation(out=pooledT[:], in_=tp[:],
                                 func=mybir.ActivationFunctionType.Copy)
            ops = finp.tile([NUM_GRAPHS, 10], F32)
            nc.tensor.matmul(out=ops[:], lhsT=pooledT[:], rhs=w_sb["W3"][:],
                             start=True, stop=False)
            nc.tensor.matmul(out=ops[:], lhsT=w_sb["ones1"][:, :NUM_GRAPHS],
                             rhs=w_sb["b3row"][:], start=False, stop=True)
            fout = fin.tile([NUM_GRAPHS, 10], F32)
            nc.scalar.activation(out=fout[:], in_=ops[:],
                                 func=mybir.ActivationFunctionType.Copy)
            nc.sync.dma_start(out=out_t[:], in_=fout[:])
        if debug:
            nc.sync.dma_start(out=dbg["dbg_h1T"][:], in_=h1T_sb[:])

    lower_extended_insts(nc)
    _split_waits(nc)
    return nc


# ---------------------------------------------------------------- entry point
def _run(x, edge_index, batch, W1_l, b1_l, W1_r, b1_r, att1, bias1,
         W2_l, b2_l, W2_r, b2_r, att2, bias2, W3, b3, ncores=NCORES,
         debug=False, trace=False):
    x = np.asarray(x, np.float32)
    per_core, meta, perm = _prep(np.asarray(x), np.asarray(edge_index),
                                 np.asarray(batch), ncores)
    consts = dict(
        W1_l=np.asarray(W1_l, np.float32).astype(BF),
        W1_r=np.asarray(W1_r, np.float32).astype(BF),
        W2_l=np.asarray(W2_l, np.float32).astype(BF),
        W2_r=np.asarray(W2_r, np.float32).astype(BF),
        att1_rep=np.tile(np.asarray(att1, np.float32).reshape(1, D),
                         (P, 1)).astype(BF),
        att2_rep=np.tile(np.asarray(att2, np.float32).reshape(1, D),
                         (P, 1)).astype(BF),
        bias1_rep=np.tile(np.asarray(bias1, np.float32).reshape(1, D),
                          (P, 1)).astype(BF),
        bias2_rep=np.tile(np.asarray(bias2, np.float32).reshape(1, D),
                          (P, 1)).astype(BF),
        b1_l=np.asarray(b1_l, np.float32).reshape(1, D).astype(BF),
        b1_r=np.asarray(b1_r, np.float32).reshape(1, D).astype(BF),
        b2_l=np.asarray(b2_l, np.float32).reshape(1, D).astype(BF),
        b2_r=np.asarray(b2_r, np.float32).reshape(1, D).astype(BF),
        iota64=np.tile(np.arange(NUM_GRAPHS, dtype=np.float32).reshape(
            1, NUM_GRAPHS), (P, 1)).astype(BF),
        W3=np.asarray(W3, np.float32).astype(BF),
        b3row=np.asarray(b3, np.float32).reshape(1, 10).astype(BF),
        ones1=np.ones((1, P), np.float32).astype(BF),
        ones_col=np.ones((P, 1), np.float32).astype(BF),
    )
    nc = _build(meta, debug=debug)
    in_maps = []
    for c in range(ncores):
        m = dict(per_core[c])
        m.update(consts)
        in_maps.append(m)
    if trace:
        _install_profhook()
    res = run_bass_kernel_spmd(nc, in_maps, core_ids=list(range(ncores)),
                               trace=trace)
    return res.results[0]["out"].astype(np.float32), (res, per_core, meta, perm)


def kernel(**inputs):
    out, _res = _run(**inputs)
    return out
